# revision 23
# baseline (speedup 1.0000x reference)
"""Trainium2 Bass kernel for nn_KeypointsLoss (v3 redesign).

Math per batch b (B=32, P=8, K=17, H=W=192, sigma=3):
    x[p,k] = trunc(kp[...,0]*191); y likewise; r/c gaussians around x/y
    target[k] = sum_p outer(r_pk, c_pk) (r vis-masked)
    loss = sum_b [sum_k |pred - target|^2] / (sum vis + 1e-6) / B

Expanded: |pred - T|^2 = S1 - 2*cross + T2 with
    S1    = sum pred^2                  (DVE affine + scalar Square, split)
    cross = sum_k <R_k^T pred_k, C_k>   (PE quad-blocked matmuls + DVE dots)
    T2    = 9*pi * sum_k sum_pp' exp(-(dx^2+dy^2)/36)   (closed form!)

Device layout (8 cores, 4 batches each):
  pred host-prepped to bf16 [128=(j4,h32), cols=(q4,hb6,w192)|k16a|k16b].
  Cross: for quad q (k=4q+j), 6 accumulating matmuls with stationary
  SD[(j,h32), (j',p)*32] block-diag gaussians -> V PSUM [64,192] per 2
  quads.  All gaussian args generated PRE-BIASED in PSUM via augmented
  broadcast matmuls (const E4/iota/ones rows appended to stationary),
  then Square (pool/DVE) + Exp (scalar) per PSUM bank.  Invisible or
  masked entries use large sentinel values -> exp underflows to exactly 0.
"""

import sys
import numpy as np

sys.path.insert(0, "/opt/trn_rl_repo")

B, P, K, H, W = 32, 8, 17, 192, 192
NCORES = 8
NB = B // NCORES            # 4 batches per core
INV2S2 = 1.0 / 18.0
RSQ2 = 0.70710678118654752  # 1/sqrt(2): folds exp(-d^2/36) into -1/18 scale
PHI2 = 28.274333882308138   # (sum_h exp(-h^2/9))^2 = 9*pi (theta corr ~0)

# pred column layout: quads 0..3 (cols (q*6+hb)*192), then k16 h<128, h>=128
CQ0, CK16A, CK16B, CTOT = 0, 4608, 4800, 4992
# XS (exp-output, bf16) column layout == X-PSUM bank layout
#   bankA: SD cols 0:512 ; bankB: SD 512:768 | cQ01 | k16aSD | Xx Xy Xx16 Xy16
#   bankC: cQ23 | cQ16 | k16bSD
XC_SD = 0
XC_CQ01 = 768
XC_K16A = 960
XC_T2X = 992
XC_T2Y = 1000
XC_T2X16 = 1008
XC_T2Y16 = 1016
XC_CQ23 = 1024
XC_CQ16 = 1216
XC_K16B = 1408
XTOT = 1440
# host-row tiles (bf16), split by contraction row-count to minimize DMA:
#   HRA [13, 768]  xseg rows (hi/lo/-2x per j + ones row12)
#   HRB [5, 688]   yc01|yc23|yc16|x16m|tklx|tkly|tkrx|tkry
#   HRC [50, 272]  t2lx|t2ly|t2rx|t2ry
HB_YC01 = 0
HB_YC23 = 128
HB_YC16 = 256
HB_X16M = 384
HB_TKLX = 416
HB_TKLY = 544
HB_TKRX = 672
HB_TKRY = 680
HBTOT = 688
HC_T2LX = 0
HC_T2LY = 128
HC_T2RX = 256
HC_T2RY = 264
HCTOT = 272
# const tile CR [13, cols]
CR_E4A = 0        # [13, 128]: per j: E4, E4, E4*h ; row12 = h^2 (h=part%32)
CR_K16A = 128     # [5, 128]: 1, 1, h, hi(h^2), lo(h^2)  (h = part)
CR_K16B = 256     # [5, 128]: same with h = part+128
CR_CQR = 384      # [5, 192]: 1, 1, -2w, hi(w^2), lo(w^2)
CRTOT = 576

S1_DVE_COLS = 2496   # DVE affine share; scalar Square gets 2496

_CACHE = {}


def _build():
    import concourse.bass as bass
    import concourse.bacc as bacc
    import concourse.tile as tile
    from concourse import mybir

    f32 = mybir.dt.float32
    bf16 = mybir.dt.bfloat16
    Alu = mybir.AluOpType
    Act = mybir.ActivationFunctionType

    nc = bacc.Bacc("TRN2", target_bir_lowering=False, debug=False,
                   num_devices=NCORES)

    fp8 = mybir.dt.float8e4
    pa_d = nc.dram_tensor("pa", [NB, 128, CTOT], fp8, kind="ExternalInput").ap()
    hx_d = nc.dram_tensor("hx", [13, NB * 768 + NB * HBTOT + CRTOT], bf16,
                          kind="ExternalInput").ap()
    hc_d = nc.dram_tensor("hc", [50, NB * HCTOT], bf16, kind="ExternalInput").ap()
    vis_d = nc.dram_tensor("visr", [1, NB * K * P], f32, kind="ExternalInput").ap()
    out_d = nc.dram_tensor("out", [1, NB], f32, kind="ExternalOutput").ap()

    with tile.TileContext(nc) as tc:
        import contextlib
        with contextlib.ExitStack() as ctx:
            consts = ctx.enter_context(tc.tile_pool(name="consts", bufs=1))
            predp = ctx.enter_context(tc.tile_pool(name="pred", bufs=1))
            genp = ctx.enter_context(tc.tile_pool(name="gen", bufs=4))
            scrp = ctx.enter_context(tc.tile_pool(name="scr", bufs=2))
            psX = ctx.enter_context(tc.tile_pool(name="psX", bufs=2, space="PSUM"))
            psV = ctx.enter_context(tc.tile_pool(name="psV", bufs=1, space="PSUM"))

            # ---- 3 small DMAs then per-b combined pred DMAs
            HXW = NB * 768 + NB * HBTOT + CRTOT
            hxa = consts.tile([13, HXW], bf16, tag="hxa", name="hxa")
            nc.sync.dma_start(out=hxa[:], in_=hx_d[:])
            hca = consts.tile([50, NB * HCTOT], bf16, tag="hca", name="hca")
            nc.sync.dma_start(out=hca[:], in_=hc_d[:])
            visr = consts.tile([1, NB * K * P], f32, tag="visr", name="visr")
            nc.sync.dma_start(out=visr[:], in_=vis_d[:])
            cr_t = hxa[:, NB * 768 + NB * HBTOT:HXW]
            pr_t = []
            for b in range(NB):
                t = predp.tile([128, CTOT], fp8, tag=f"pr{b}", name=f"pr{b}")
                (nc.sync, nc.scalar, nc.sync, nc.scalar)[b].dma_start(
                    out=t[:], in_=pa_d[b])
                pr_t.append(t)

            onesc = consts.tile([128, 1], f32, tag="onesc", name="onesc")
            nc.vector.memset(onesc[:], 1.0)
            visr = consts.tile([1, NB * K * P], f32, tag="visr", name="visr")
            nc.sync.dma_start(out=visr[:], in_=vis_d[:])

            acc = consts.tile([128, 32], f32, tag="acc", name="acc")
            nc.vector.memset(acc[:], 0.0)

            # phase 1: gens + Exps + T2 (pred-independent)
            xs_t = []
            for b in range(NB):
                ha = hxa[:, b * 768:(b + 1) * 768]
                hb_ = hxa[:, NB * 768 + b * HBTOT:NB * 768 + (b + 1) * HBTOT]
                hc = hca[:, b * HCTOT:(b + 1) * HCTOT]
                # ---- X-gen matmuls into 3 PSUM banks (all pre-biased)
                bA = psX.tile([128, 512], f32, tag="bA", name=f"bA{b}")
                bB = psX.tile([128, 512], f32, tag="bB", name=f"bB{b}")
                bC = psX.tile([128, 416], f32, tag="bC", name=f"bC{b}")
                # SD main (emits (x-h)^2): lhsT = E4aug [13,128]
                nc.tensor.matmul(bA[:, 0:512], cr_t[0:13, CR_E4A:CR_E4A + 128],
                                 ha[0:13, 0:512],
                                 start=True, stop=True)
                nc.tensor.matmul(bB[:, 0:256], cr_t[0:13, CR_E4A:CR_E4A + 128],
                                 ha[0:13, 512:768],
                                 start=True, stop=True)
                # cQ01 / cQ23 / cQ16 ((w-y)^2): lhsT = y-rows, rhs = cqrhs
                nc.tensor.matmul(bB[:, 256:448], hb_[0:5, HB_YC01:HB_YC01 + 128],
                                 cr_t[0:5, CR_CQR:CR_CQR + 192],
                                 start=True, stop=True)
                nc.tensor.matmul(bC[:, 0:192], hb_[0:5, HB_YC23:HB_YC23 + 128],
                                 cr_t[0:5, CR_CQR:CR_CQR + 192],
                                 start=True, stop=True)
                nc.tensor.matmul(bC[:, 192:384], hb_[0:5, HB_YC16:HB_YC16 + 128],
                                 cr_t[0:5, CR_CQR:CR_CQR + 192],
                                 start=True, stop=True)
                # k16 SD a/b ((x16-h)^2)
                nc.tensor.matmul(bB[:, 448:480], cr_t[0:5, CR_K16A:CR_K16A + 128],
                                 hb_[0:5, HB_X16M:HB_X16M + 32],
                                 start=True, stop=True)
                nc.tensor.matmul(bC[:, 384:416], cr_t[0:5, CR_K16B:CR_K16B + 128],
                                 hb_[0:5, HB_X16M:HB_X16M + 32],
                                 start=True, stop=True)
                # T2 args (dx^2/2) [128,8] ; k16 versions
                nc.tensor.matmul(bB[:, 480:488], hc[0:50, HC_T2LX:HC_T2LX + 128],
                                 hc[0:50, HC_T2RX:HC_T2RX + 8],
                                 start=True, stop=True)
                nc.tensor.matmul(bB[:, 488:496], hc[0:50, HC_T2LY:HC_T2LY + 128],
                                 hc[0:50, HC_T2RY:HC_T2RY + 8],
                                 start=True, stop=True)
                nc.tensor.matmul(bB[:, 496:504], hb_[0:5, HB_TKLX:HB_TKLX + 128],
                                 hb_[0:5, HB_TKRX:HB_TKRX + 8],
                                 start=True, stop=True)
                nc.tensor.matmul(bB[:, 504:512], hb_[0:5, HB_TKLY:HB_TKLY + 128],
                                 hb_[0:5, HB_TKRY:HB_TKRY + 8],
                                 start=True, stop=True)

                # ---- Exp (scalar, PSUM src: banks already hold ARG^2)
                xs = genp.tile([128, XTOT], bf16, tag="xs", name=f"xs{b}")
                nc.scalar.activation(xs[:, 0:512], bA[:], Act.Exp,
                                     scale=-INV2S2)
                nc.scalar.activation(xs[:, 512:1024], bB[:], Act.Exp,
                                     scale=-INV2S2)
                nc.scalar.activation(xs[:, 1024:1440], bC[:], Act.Exp,
                                     scale=-INV2S2)
                xs_t.append(xs)

            # phase 2: cross + dots + S1 (pred-gated)
            for b in range(NB):
                xs = xs_t[b]
                pr = pr_t[b]
                pa = pr[:, 0:S1_DVE_COLS]
                pb = pr[:, S1_DVE_COLS:CTOT]
                # ---- cross matmuls: quads into V01/V23, k16 into V16
                v01 = psV.tile([64, 192], f32, tag="v01", name=f"v01{b}")
                vcmb = psV.tile([64, 384], f32, tag="vcmb", name=f"vcmb{b}")
                v23 = vcmb[:, 0:192]
                v16 = vcmb[0:32, 192:384]
                for hb in range(6):
                    for q in (0, 2, 1, 3):   # alternate PSUM banks
                        vt = v01 if q < 2 else v23
                        r0 = 32 * (q % 2)
                        col = (q * 6 + hb) * 32
                        pc = (q * 6 + hb) * 192
                        nc.tensor.matmul(
                            vt[r0:r0 + 32, :],
                            xs[:, col:col + 32],
                            pr[:, pc:pc + 192],
                            start=hb == 0, stop=hb == 5)
                # k16: contraction h 0:128 then 128:192
                nc.tensor.matmul(v16, xs[:, XC_K16A:XC_K16A + 32],
                                 pr[:, CK16A:CK16A + 192],
                                 start=True, stop=False)
                nc.tensor.matmul(v16, xs[0:64, XC_K16B:XC_K16B + 32],
                                 pr[0:64, CK16B:CK16B + 192],
                                 start=False, stop=True)

                # ---- S1 DVE (pred-only, earliest ready)
                s1a = scrp.tile([128, S1_DVE_COLS], bf16, tag="s1a", name=f"s1a{b}")
                nc.vector.affine_mul_reduce(
                    out=s1a[:], accum_out=acc[:, 0 + b:1 + b],
                    in0=pa[:], in1=pa[:], scale=1.0, bias=0.0)
                # ---- DVE dots (scale -2 folds the cross sign)
                d01 = scrp.tile([64, 192], bf16, tag="d01", name=f"d01{b}")
                nc.vector.affine_mul_reduce(
                    out=d01[:], accum_out=acc[0:64, 16 + b:17 + b],
                    in0=v01[:], in1=xs[0:64, XC_CQ01:XC_CQ01 + 192],
                    scale=-2.0, bias=0.0)
                d23 = scrp.tile([64, 192], bf16, tag="d23", name=f"d23{b}")
                nc.vector.affine_mul_reduce(
                    out=d23[:], accum_out=acc[0:64, 20 + b:21 + b],
                    in0=v23, in1=xs[0:64, XC_CQ23:XC_CQ23 + 192],
                    scale=-2.0, bias=0.0)
                d16 = scrp.tile([32, 192], bf16, tag="d16", name=f"d16{b}")
                nc.vector.affine_mul_reduce(
                    out=d16[:], accum_out=acc[0:32, 24 + b:25 + b],
                    in0=v16, in1=xs[0:32, XC_CQ16:XC_CQ16 + 192],
                    scale=-2.0, bias=0.0)
                # ---- T2 affines
                t2s = scrp.tile([128, 16], bf16, tag="t2s", name=f"t2s{b}")
                nc.vector.affine_mul_reduce(
                    out=t2s[:, 0:8], accum_out=acc[:, 8 + b:9 + b],
                    in0=xs[:, XC_T2X:XC_T2X + 8], in1=xs[:, XC_T2Y:XC_T2Y + 8],
                    scale=PHI2, bias=0.0)
                nc.vector.affine_mul_reduce(
                    out=t2s[:, 8:16], accum_out=acc[:, 12 + b:13 + b],
                    in0=xs[:, XC_T2X16:XC_T2X16 + 8],
                    in1=xs[:, XC_T2Y16:XC_T2Y16 + 8],
                    scale=PHI2, bias=0.0)

            # ---- S1 scalar squares last (gate only the final reduce)
            for b in range(NB):
                s1b = scrp.tile([128, CTOT - S1_DVE_COLS], bf16, tag="s1b",
                                name=f"s1b{b}")
                nc.scalar.activation(s1b[:], pr_t[b][:, S1_DVE_COLS:CTOT],
                                     Act.Square,
                                     accum_out=acc[:, 4 + b:5 + b])

            # ---- finalize: partition-reduce acc via ones matmul (fp32)
            vfin = psV.tile([64, 384], f32, tag="vcmb", name="vfin")
            finp = vfin[0:1, 0:32]
            nc.tensor.matmul(finp, onesc[:], acc[:], start=True, stop=True)
            per = consts.tile([1, NB], f32, tag="per", name="per")
            nc.vector.tensor_reduce(
                per[:][:, :, None],
                finp.rearrange("p (g b) -> p b g", b=NB),
                axis=mybir.AxisListType.X, op=Alu.add)
            den = consts.tile([1, NB], f32, tag="den", name="den")
            nc.vector.tensor_reduce(
                den[:][:, :, None],
                visr[:].rearrange("p (b c) -> p b c", c=K * P),
                axis=mybir.AxisListType.X, op=Alu.add)
            nc.vector.tensor_scalar_add(den[:], den[:], 1e-6)
            invd = consts.tile([1, NB], f32, tag="invd", name="invd")
            nc.vector.reciprocal(invd[:], den[:])
            outt = consts.tile([1, NB], f32, tag="outt", name="outt")
            nc.vector.tensor_tensor(outt[:], per[:], invd[:], Alu.mult)
            nc.sync.dma_start(out=out_d[:], in_=outt[:])

    nc.compile()
    return nc


def get_nc():
    if "nc" not in _CACHE:
        _CACHE["nc"] = _build()
    return _CACHE["nc"]


def _consts():
    import ml_dtypes
    bf = ml_dtypes.bfloat16
    cr = np.zeros((13, CRTOT), dtype=np.float32)
    hl = np.tile(np.arange(32, dtype=np.float32), 4)      # h-local per part
    for j in range(4):
        e4 = np.zeros(128, dtype=np.float32)
        e4[32 * j:32 * j + 32] = 1.0
        cr[3 * j + 0, CR_E4A:CR_E4A + 128] = e4
        cr[3 * j + 1, CR_E4A:CR_E4A + 128] = e4
        cr[3 * j + 2, CR_E4A:CR_E4A + 128] = e4 * hl
    cr[12, CR_E4A:CR_E4A + 128] = hl * hl                 # h^2 <= 961 exact
    for base, h0 in ((CR_K16A, 0.0), (CR_K16B, 128.0)):
        h = np.arange(128, dtype=np.float32) + h0
        hi, lo = _hilo(h * h)
        cr[0, base:base + 128] = 1.0
        cr[1, base:base + 128] = 1.0
        cr[2, base:base + 128] = h
        cr[3, base:base + 128] = hi
        cr[4, base:base + 128] = lo
    w = np.arange(192, dtype=np.float32)
    whi, wlo = _hilo(w * w)
    cr[0, CR_CQR:CR_CQR + 192] = 1.0
    cr[1, CR_CQR:CR_CQR + 192] = 1.0
    cr[2, CR_CQR:CR_CQR + 192] = -2.0 * w
    cr[3, CR_CQR:CR_CQR + 192] = whi
    cr[4, CR_CQR:CR_CQR + 192] = wlo
    return cr.astype(bf)


def _hilo(v):
    """Split f32 array into hi=bf16(v) and lo=bf16(v-hi) rows."""
    import ml_dtypes
    bf = ml_dtypes.bfloat16
    hi = v.astype(bf).astype(np.float32)
    lo = (v - hi).astype(bf).astype(np.float32)
    return hi, lo


def _host_rows(x, y, valid):
    """x,y: [NB,K,P] f32 trunc coords; valid: bool. Returns HR [NB,50,HRTOT].

    All masked/raw coordinate bases are rounded to bf16 FIRST, then the
    squared hi/lo rows derive from the rounded value, so (a-b)^2 structure
    is preserved exactly and masked pairs stay huge."""
    import ml_dtypes
    bf = ml_dtypes.bfloat16
    nb = x.shape[0]

    def r(v):
        return v.astype(bf).astype(np.float32)

    idx = (np.arange(K * P, dtype=np.float32)).reshape(K, P)
    rowmask = 20000.0 + 256.0 * idx
    colmask = 45000.0 + 256.0 * idx
    xm = r(np.where(valid, x, rowmask[None]))
    xc = r(np.where(valid, x, colmask[None]))
    ym = r(np.where(valid, y, rowmask[None]))
    yc_ = r(np.where(valid, y, colmask[None]))
    ha = np.zeros((nb, 13, 768), dtype=np.float32)
    hb = np.zeros((nb, 5, HBTOT), dtype=np.float32)
    hc = np.zeros((nb, 50, HCTOT), dtype=np.float32)
    # xseg [13, 768]
    for q in range(4):
        for hbk in range(6):
            base = (q * 6 + hbk) * 32
            for j in range(4):
                xt = r(xm[:, 4 * q + j, :] - 32.0 * hbk)
                hi, lo = _hilo(xt * xt)
                ha[:, 3 * j + 0, base:base + 32] = 4e8
                ha[:, 3 * j + 0, base + 8 * j:base + 8 * j + 8] = hi
                ha[:, 3 * j + 1, base + 8 * j:base + 8 * j + 8] = lo
                ha[:, 3 * j + 2, base + 8 * j:base + 8 * j + 8] = -2.0 * xt
    ha[:, 12, :] = 1.0
    # yc tiles [5, 128]
    for base, qs in ((HB_YC01, (0, 1)), (HB_YC23, (2, 3))):
        col = np.full((nb, 128), 20000.0, dtype=np.float32)
        for qi, q in enumerate(qs):
            for j in range(4):
                col[:, 32 * qi + 8 * j:32 * qi + 8 * j + 8] = ym[:, 4 * q + j, :]
        hi, lo = _hilo(col * col)
        hb[:, 0, base:base + 128] = hi
        hb[:, 1, base:base + 128] = lo
        hb[:, 2, base:base + 128] = col
        hb[:, 3, base:base + 128] = 1.0
        hb[:, 4, base:base + 128] = 1.0
    col = np.full((nb, 128), 20000.0, dtype=np.float32)
    col[:, 0:8] = ym[:, 16, :]
    hi, lo = _hilo(col * col)
    hb[:, 0, HB_YC16:HB_YC16 + 128] = hi
    hb[:, 1, HB_YC16:HB_YC16 + 128] = lo
    hb[:, 2, HB_YC16:HB_YC16 + 128] = col
    hb[:, 3, HB_YC16:HB_YC16 + 128] = 1.0
    hb[:, 4, HB_YC16:HB_YC16 + 128] = 1.0
    # x16m rhs [5, 32]
    xv = np.zeros((nb, 32), dtype=np.float32)
    xv[:, 0:8] = xm[:, 16, :]
    hi, lo = _hilo(xv * xv)
    hb[:, 0, HB_X16M:HB_X16M + 32] = 4e8
    hb[:, 0, HB_X16M:HB_X16M + 8] = hi[:, 0:8]
    hb[:, 1, HB_X16M:HB_X16M + 8] = lo[:, 0:8]
    hb[:, 2, HB_X16M:HB_X16M + 8] = -2.0 * xv[:, 0:8]
    hb[:, 3, HB_X16M:HB_X16M + 32] = 1.0
    hb[:, 4, HB_X16M:HB_X16M + 32] = 1.0
    # T2 main [50, 128]+[50, 8]
    for base_l, base_r, vrow, vcol in (
            (HC_T2LX, HC_T2RX, xm, xc), (HC_T2LY, HC_T2RY, ym, yc_)):
        vr128 = vrow[:, 0:16, :].reshape(nb, 128)
        for jk in range(16):
            e = np.zeros((nb, 128), dtype=np.float32)
            e[:, 8 * jk:8 * jk + 8] = 1.0
            hc[:, 3 * jk + 0, base_l:base_l + 128] = e
            hc[:, 3 * jk + 1, base_l:base_l + 128] = e
            hc[:, 3 * jk + 2, base_l:base_l + 128] = e * vr128
            chi, clo = _hilo(0.5 * vcol[:, jk, :] ** 2)
            hc[:, 3 * jk + 0, base_r:base_r + 8] = chi
            hc[:, 3 * jk + 1, base_r:base_r + 8] = clo
            hc[:, 3 * jk + 2, base_r:base_r + 8] = -vcol[:, jk, :]
        rhi, rlo = _hilo(0.5 * vr128 * vr128)
        hc[:, 48, base_l:base_l + 128] = rhi
        hc[:, 49, base_l:base_l + 128] = rlo
        hc[:, 48, base_r:base_r + 8] = 1.0
        hc[:, 49, base_r:base_r + 8] = 1.0
    # T2 k16 [5, 128]+[5, 8]
    for base_l, base_r, vrow, vcol in (
            (HB_TKLX, HB_TKRX, xm, xc), (HB_TKLY, HB_TKRY, ym, yc_)):
        colp = np.full((nb, 128), 20000.0, dtype=np.float32)
        colp[:, 0:8] = vrow[:, 16, :]
        rhi, rlo = _hilo(0.5 * colp * colp)
        hb[:, 0, base_l:base_l + 128] = rhi
        hb[:, 1, base_l:base_l + 128] = rlo
        hb[:, 2, base_l:base_l + 128] = colp
        hb[:, 3, base_l:base_l + 128] = 1.0
        hb[:, 4, base_l:base_l + 128] = 1.0
        chi, clo = _hilo(0.5 * vcol[:, 16, :] ** 2)
        hb[:, 0, base_r:base_r + 8] = 1.0
        hb[:, 1, base_r:base_r + 8] = 1.0
        hb[:, 2, base_r:base_r + 8] = -vcol[:, 16, :]
        hb[:, 3, base_r:base_r + 8] = chi
        hb[:, 4, base_r:base_r + 8] = clo
    return ha.astype(bf), hb.astype(bf), hc.astype(bf)


def make_in_maps(pred_heatmaps, keypoints, visibilities):
    import ml_dtypes
    bf = ml_dtypes.bfloat16
    pred = np.asarray(pred_heatmaps, dtype=np.float32)
    kp = np.asarray(keypoints, dtype=np.float32)
    vis = np.asarray(visibilities, dtype=np.int32)

    x = np.trunc(kp[..., 0] * (W - 1)).transpose(0, 2, 1)  # [B,K,P]
    y = np.trunc(kp[..., 1] * (H - 1)).transpose(0, 2, 1)
    visk = vis.transpose(0, 2, 1)                           # [B,K,P]
    valid = (visk > 0) & (x >= 0) & (x < W) & (y >= 0) & (y < H)

    # pred -> PA [B, 128, 4992] bf16
    A = pred[:, :16].reshape(B, 4, 4, 6, 32, 192).transpose(0, 2, 4, 1, 3, 5)
    A = A.reshape(B, 128, 4608)
    k16a = pred[:, 16, 0:128, :]
    k16b = np.zeros((B, 128, 192), dtype=np.float32)
    k16b[:, 0:64] = pred[:, 16, 128:192, :]
    PA = np.concatenate([A, k16a, k16b], axis=2).astype(
        ml_dtypes.float8_e4m3fn)

    cr = _consts()
    in_maps = []
    for c in range(NCORES):
        sl = slice(c * NB, (c + 1) * NB)
        ha, hb, hc = _host_rows(x[sl], y[sl], valid[sl])
        hx = np.zeros((13, NB * 768 + NB * HBTOT + CRTOT), dtype=ha.dtype)
        hx[:, 0:NB * 768] = ha.transpose(1, 0, 2).reshape(13, -1)
        hx[0:5, NB * 768:NB * 768 + NB * HBTOT] = \
            hb.transpose(1, 0, 2).reshape(5, -1)
        hx[:, NB * 768 + NB * HBTOT:] = cr
        in_maps.append({
            "pa": np.ascontiguousarray(PA[sl]),
            "hx": hx,
            "hc": np.ascontiguousarray(hc.transpose(1, 0, 2).reshape(50, -1)),
            "visr": np.ascontiguousarray(
                visk[sl].astype(np.float32).reshape(1, NB * K * P)),
        })
    return in_maps


def kernel(pred_heatmaps, keypoints, visibilities):
    from concourse.bass_utils import run_bass_kernel_spmd

    nc = get_nc()
    in_maps = make_in_maps(pred_heatmaps, keypoints, visibilities)
    res = run_bass_kernel_spmd(nc, in_maps, core_ids=list(range(NCORES)))
    total = np.float64(0.0)
    for c in range(NCORES):
        total += np.asarray(res.results[c]["out"], dtype=np.float64).sum()
    return np.float32(total / B)
